# revision 3
# baseline (speedup 1.0000x reference)
"""Trainium2 Bass kernel v2 for AttLayer-style attention pooling.

Computes, for x[B, T, D], W[D, A], b[A], u[A, 1]:
    uit = tanh(x @ W + b)            # [B, T, A]
    z   = uit @ u[:, 0]              # [B, T]
    e   = exp(z)
    a   = e / (sum_t e + 1e-7)
    y   = einsum('btd,bt->bd', x, a) # [B, D]

Sharding: pure data parallel over batch; 8 cores x 8 batches each.

v2 structure (vs v1): instruction-minimal bf16 dataflow.
  1. SWDGE cast-DMA loads 2 batches of x as bf16 [128, 2, 16, 256]
     (partition p holds rows t = p*16 + i).
  2. ONE xbar DMA-transpose per batch builds xT [dl, (i ch), cc] in a
     single instruction (in [128, 4096] -> out [128, 32, 128]).
  3. mm1: W-chunk stationary, xT moving in 512-col groups -> uitT in
     PSUM; ScalarE tanh(+b) -> SBUF bf16.  8 matmuls + 4 acts per batch.
  4. mm2: uitT 128-col chunks stationary vs u -> z[t-part, i] in PSUM;
     ScalarE exp (accum_out) writes e into batch-column bi of a zeroed
     [128, I, BC] pad tile, and row sums into column bi of s_pad.
  5. mm3: e_pad[:, i, :] stationary ([128, BC], zeros except col bi) vs
     x_nat moving accumulates ALL batches into one [BC, D] PSUM tile
     (row per batch).  One ones-matmul folds s_pad -> S[BC, 1].
  6. Single normalize (add eps, reciprocal, scale) + ONE output DMA.
"""

from contextlib import ExitStack

import numpy as np

import concourse.bass as bass
import concourse.tile as tile
from concourse import mybir
from concourse.bass_utils import run_bass_kernel_spmd

N_CORES = 8
B, T, D, A = 64, 2048, 256, 128
BC = B // N_CORES  # batches per core
I = T // 128  # partition p holds t = p*I + i
EPS = 1e-7

F32 = mybir.dt.float32
BF16 = mybir.dt.bfloat16
TANH = mybir.ActivationFunctionType.Tanh
EXP = mybir.ActivationFunctionType.Exp

LOAD_PLAN = (1, 1, 2, 2, 2)  # batches per cast-DMA load (progressive warmup)
TR_CHUNK = 4  # i-rows per DMA-transpose instruction (16 = whole batch)
TRANSPOSE = "pe"  # "xbar" (DMA) or "pe" (TensorE transpose + copy)


def _split_multi_waits(nc):
    """Hoist all-but-one sem wait off instructions onto no-ops.

    The walrus build in this container rejects instructions carrying
    more than one sync-wait command (CoreV3 setupSyncWait). A no-op on
    the same engine immediately before the instruction is semantically
    identical: the engine blocks on each wait in sequence.
    """
    counter = [0]

    def fresh_nop(engine, wait):
        counter[0] += 1
        n = mybir.InstNoOp(name=f"I-waitsplit-{counter[0]}", ins=[], outs=[])
        n.engine = engine
        n.sync_info = mybir.SyncInfo(on_wait=[wait], on_update=[])
        nc.register_instruction(n)
        return n

    for fn in nc.m.functions:
        for blk in fn.blocks:
            changed = False
            out = []
            for inst in blk.instructions:
                si = inst.sync_info
                if si is not None and si.on_wait and len(si.on_wait) > 1:
                    waits = list(si.on_wait)
                    for w in waits[:-1]:
                        out.append(fresh_nop(inst.engine, w))
                    si.on_wait = waits[-1:]
                    changed = True
                out.append(inst)
            if changed:
                blk.instructions = out


def _emit_body(ctx, tc, x, W, b, u, out, repeat=1):
    nc = tc.nc

    singles = ctx.enter_context(tc.tile_pool(name="singles", bufs=1))
    xpool = ctx.enter_context(tc.tile_pool(name="xnat", bufs=3))
    xtpool = ctx.enter_context(tc.tile_pool(name="xt", bufs=3))
    upool = ctx.enter_context(tc.tile_pool(name="uit", bufs=2))
    spool = ctx.enter_context(tc.tile_pool(name="small", bufs=2))
    pu_pool = ctx.enter_context(tc.tile_pool(name="pu", bufs=2, space="PSUM"))
    pa_pool = ctx.enter_context(tc.tile_pool(name="pa", bufs=2, space="PSUM"))
    py_bufs = 1 if TRANSPOSE == "pe" else 2
    py_pool = ctx.enter_context(tc.tile_pool(name="py", bufs=py_bufs, space="PSUM"))
    ps_pool = ctx.enter_context(tc.tile_pool(name="ps", bufs=1, space="PSUM"))
    if TRANSPOSE == "pe":
        tr_pool = ctx.enter_context(tc.tile_pool(name="tr", bufs=2, space="PSUM"))
    else:
        tr_pool = None

    # Replicated parameters on HWDGE (keeps the gpsimd SWDGE queue free
    # for the x cast-loads). W is consumed as two [128, A] K-chunks.
    W_f = singles.tile([128, 2, A], F32)
    nc.sync.dma_start(W_f[:], W.ap().rearrange("(c k) a -> k c a", c=2))
    W_bf = singles.tile([128, 2, A], BF16)
    nc.vector.tensor_copy(W_bf[:], W_f[:])
    b_sb = singles.tile([A, 1], F32)
    nc.sync.dma_start(b_sb[:], b.ap().rearrange("(a o) -> a o", o=1))
    u_f = singles.tile([A, 1], F32)
    nc.sync.dma_start(u_f[:], u.ap())
    u_bf = singles.tile([A, 1], BF16)
    nc.vector.tensor_copy(u_bf[:], u_f[:])
    ones_f = singles.tile([128, 1], F32)
    nc.vector.memset(ones_f[:], 1.0)
    if TRANSPOSE == "pe":
        from concourse.masks import make_identity

        identity = singles.tile([128, 128], BF16)
        make_identity(nc, identity[:])
    else:
        identity = None

    def one_pass():
        pys = py_pool.tile([BC, 512], F32, tag="py")
        s_pad = spool.tile([128, BC], F32, tag="spad")
        nc.vector.memset(s_pad[:], 0.0)
        b0 = 0
        for nb in LOAD_PLAN:
            xg = xpool.tile([128, nb, I, D], BF16, tag=f"xg{nb}")
            nc.gpsimd.dma_start(
                xg[:],
                x.ap()[b0 : b0 + nb].rearrange("bb (p i) d -> p bb i d", i=I),
            )
            for j in range(nb):
                _emit_batch(tc, pools, params, b0 + j, xg[:, j], pys, s_pad)
            b0 += nb

        # S[b] = sum_p s_pad[p, b] for all batches at once.
        ps8 = ps_pool.tile([BC, 1], F32, tag="ps")
        nc.tensor.matmul(ps8[:], s_pad[:], ones_f[:], start=True, stop=True)

        # y = y' / (S + eps), one DMA out for all batches.
        r_all = spool.tile([BC, 1], F32, tag="rall")
        nc.vector.tensor_scalar_add(r_all[:], ps8[:], EPS)
        nc.vector.reciprocal(r_all[:], r_all[:])
        y_n = spool.tile([BC, D], F32, tag="ynorm")
        nc.vector.tensor_scalar_mul(y_n[:], pys[:, 0:D], r_all[:])
        nc.sync.dma_start(out.ap(), y_n[:])

    pools = (xpool, xtpool, upool, spool, pu_pool, pa_pool, tr_pool)
    params = (W_bf, b_sb, u_bf, ones_f, identity)
    for _ in range(repeat):
        one_pass()


def _emit_batch(tc, pools, params, bi, x_nat, pys, s_pad):
    nc = tc.nc
    (xpool, xtpool, upool, spool, pu_pool, pa_pool, tr_pool) = pools
    (W_bf, b_sb, u_bf, ones_f, identity) = params

    # Transpose: xt[dl, i, ch, cc] = x[t = cc*I + i, d = ch*128 + dl].
    # One tile per chunk so mm1 group g depends only on chunk g.
    n_tr = I // TR_CHUNK
    xts = []
    for tg in range(n_tr):
        xtc = xtpool.tile([128, TR_CHUNK, 2, 128], BF16, tag=f"xt{tg}")
        sl = slice(tg * TR_CHUNK, (tg + 1) * TR_CHUNK)
        if TRANSPOSE == "xbar":
            nc.sync.dma_start(xtc[:], x_nat[:, sl, :], transpose=True)
        else:
            pt = tr_pool.tile([128, TR_CHUNK, 2, 128], BF16, tag="tr")
            for il in range(TR_CHUNK):
                for ch in (0, 1):
                    nc.tensor.transpose(
                        pt[:, il, ch, :],
                        x_nat[:, tg * TR_CHUNK + il, 128 * ch : 128 * (ch + 1)],
                        identity[:],
                    )
            nc.vector.tensor_copy(xtc[:], pt[:])
        xts.append(xtc)

    # mm1 + tanh: uitT[a, i, cc] = tanh(sum_d W[d, a] xT[d, (i cc)] + b[a])
    gi = 4 // TR_CHUNK if TR_CHUNK < 4 else 1  # chunks per 4-i matmul group
    uits = []
    for g in range(I // 4):
        pug = pu_pool.tile([A, 4, 128], F32, tag="pu")
        for ch in (0, 1):
            if TR_CHUNK >= 4:
                xt_g = xts[4 * g // TR_CHUNK]
                off = (4 * g) % TR_CHUNK
                mv = xt_g[:, off : off + 4, ch, :]
                nc.tensor.matmul(
                    pug[:], W_bf[:, ch, :], mv, start=(ch == 0), stop=(ch == 1)
                )
            else:
                for q in range(gi):
                    nc.tensor.matmul(
                        pug[:, q * TR_CHUNK : (q + 1) * TR_CHUNK, :],
                        W_bf[:, ch, :],
                        xts[4 * g // TR_CHUNK + q][:, :, ch, :],
                        start=(ch == 0),
                        stop=(ch == 1),
                    )
        uitg = upool.tile([A, 4, 128], BF16, tag=f"uit{g}")
        nc.scalar.activation(uitg[:], pug[:], TANH, bias=b_sb[:])
        uits.append(uitg)

    # mm2: z[t-part, i] = sum_a uitT[a, i, t-part] * u[a]
    pait = pa_pool.tile([128, I], F32, tag="pa")
    for i in range(I):
        nc.tensor.matmul(
            pait[:, i : i + 1],
            uits[i // 4][:, i % 4, :],
            u_bf[:],
            start=True,
            stop=True,
        )

    # exp -> batch-column bi of a zeroed pad tile (bf16), with fused
    # per-partition row sums into column bi of s_pad (f32).
    e_pad = spool.tile([128, I, BC], BF16, tag="epad")
    nc.vector.memset(e_pad[:], 0.0)
    nc.scalar.activation(
        e_pad[:, :, bi], pait[:], EXP, accum_out=s_pad[:, bi : bi + 1]
    )

    # mm3: row bi of pys accumulates y'_bi = sum_t e[t] x[t, :] (other
    # stationary columns are zero, so other rows get +0).
    for i in range(I):
        nc.tensor.matmul(
            pys[:, 0:D],
            e_pad[:, i, :],
            x_nat[:, i, :],
            start=(bi == 0 and i == 0),
            stop=(bi == BC - 1 and i == I - 1),
        )


_NC_CACHE = {}


def _build_nc(repeat=1, hw_loop=False):
    key = (repeat, hw_loop)
    if key in _NC_CACHE:
        return _NC_CACHE[key]
    nc = bass.Bass()
    x = nc.declare_dram_parameter("x", [BC, T, D], F32, isOutput=False)
    W = nc.declare_dram_parameter("W", [D, A], F32, isOutput=False)
    b = nc.declare_dram_parameter("b", [A], F32, isOutput=False)
    u = nc.declare_dram_parameter("u", [A, 1], F32, isOutput=False)
    out = nc.declare_dram_parameter("out", [BC, D], F32, isOutput=True)
    with tile.TileContext(nc) as tc, ExitStack() as ctx:
        _emit_body(ctx, tc, x, W, b, u, out, repeat=repeat)
    _split_multi_waits(nc)
    _NC_CACHE[key] = nc
    return nc


def make_in_maps(x, W, b, u):
    x = np.ascontiguousarray(x, dtype=np.float32)
    W = np.ascontiguousarray(W, dtype=np.float32)
    b = np.ascontiguousarray(b, dtype=np.float32)
    u = np.ascontiguousarray(u, dtype=np.float32)
    return [
        {"x": x[c * BC : (c + 1) * BC], "W": W, "b": b, "u": u}
        for c in range(N_CORES)
    ]


def kernel(x, W, b, u):
    nc = _build_nc()
    res = run_bass_kernel_spmd(nc, make_in_maps(x, W, b, u), list(range(N_CORES)))
    return np.concatenate([r["out"] for r in res.results], axis=0)


# revision 4
# speedup vs baseline: 1.1016x; 1.1016x over previous
"""Trainium2 Bass kernel v2 for AttLayer-style attention pooling.

Computes, for x[B, T, D], W[D, A], b[A], u[A, 1]:
    uit = tanh(x @ W + b)            # [B, T, A]
    z   = uit @ u[:, 0]              # [B, T]
    e   = exp(z)
    a   = e / (sum_t e + 1e-7)
    y   = einsum('btd,bt->bd', x, a) # [B, D]

Sharding: pure data parallel over batch; 8 cores x 8 batches each.

v2 structure (vs v1): minimal-HBM bf16 dataflow, all batches pooled
into one PSUM tile, one output DMA.
  1. SWDGE cast-DMA loads x as bf16 [128, nb, 16, 256] in progressive
     chunks (1,1,2,2,2 batches) so compute starts ~5us in; partition p
     holds rows t = p*16 + i.  HBM read traffic stays at the 16 MiB
     floor; params go over HWDGE to keep the SWDGE queue clear.
  2. TensorE transposes ([128,128] tiles via identity) build xT chunks
     [dl, i, ch, cc]; one DVE copy per 4-i chunk drains PSUM->SBUF.
     (TRANSPOSE="xbar" swaps these for DMA-transposes: fewer
     instructions but 16 MiB extra fabric traffic and slow DmaTranspose
     ucode on the measured path.)
  3. mm1: W-chunk stationary, xT moving in 512-col groups -> uitT in
     PSUM; ScalarE tanh(+b) -> SBUF bf16.  8 matmuls + 4 acts per batch.
  4. mm2: uitT 128-col chunks stationary vs u -> z[t-part, i] in PSUM;
     ScalarE exp (accum_out) writes e into batch-column bi of a zeroed
     [128, I, BC] pad tile, and row sums into column bi of s_pad.
  5. mm3: e_pad[:, i, :] stationary ([128, BC], zeros except col bi) vs
     x_nat moving accumulates ALL batches into one [BC, D] PSUM tile
     (row per batch; rows land partition-aligned so no per-batch copies
     or unaligned partition writes).  One ones-matmul folds s_pad ->
     S[BC, 1] for every batch at once.
  6. Single normalize (add eps, reciprocal, scale) + ONE output DMA.
"""

from contextlib import ExitStack

import numpy as np

import concourse.bass as bass
import concourse.tile as tile
from concourse import mybir
from concourse.bass_utils import run_bass_kernel_spmd

N_CORES = 8
B, T, D, A = 64, 2048, 256, 128
BC = B // N_CORES  # batches per core
I = T // 128  # partition p holds t = p*I + i
EPS = 1e-7

F32 = mybir.dt.float32
BF16 = mybir.dt.bfloat16
TANH = mybir.ActivationFunctionType.Tanh
EXP = mybir.ActivationFunctionType.Exp

LOAD_PLAN = (1, 1, 2, 2, 2)  # batches per cast-DMA load (progressive warmup)
TR_CHUNK = 4  # i-rows per DMA-transpose instruction (16 = whole batch)
TRANSPOSE = "pe"  # "xbar" (DMA) or "pe" (TensorE transpose + copy)


def _split_multi_waits(nc):
    """Hoist all-but-one sem wait off instructions onto no-ops.

    The walrus build in this container rejects instructions carrying
    more than one sync-wait command (CoreV3 setupSyncWait). A no-op on
    the same engine immediately before the instruction is semantically
    identical: the engine blocks on each wait in sequence.
    """
    counter = [0]

    def fresh_nop(engine, wait):
        counter[0] += 1
        n = mybir.InstNoOp(name=f"I-waitsplit-{counter[0]}", ins=[], outs=[])
        n.engine = engine
        n.sync_info = mybir.SyncInfo(on_wait=[wait], on_update=[])
        nc.register_instruction(n)
        return n

    for fn in nc.m.functions:
        for blk in fn.blocks:
            changed = False
            out = []
            for inst in blk.instructions:
                si = inst.sync_info
                if si is not None and si.on_wait and len(si.on_wait) > 1:
                    waits = list(si.on_wait)
                    for w in waits[:-1]:
                        out.append(fresh_nop(inst.engine, w))
                    si.on_wait = waits[-1:]
                    changed = True
                out.append(inst)
            if changed:
                blk.instructions = out


def _emit_body(ctx, tc, x, W, b, u, out, repeat=1):
    nc = tc.nc

    singles = ctx.enter_context(tc.tile_pool(name="singles", bufs=1))
    xpool = ctx.enter_context(tc.tile_pool(name="xnat", bufs=3))
    xtpool = ctx.enter_context(tc.tile_pool(name="xt", bufs=3))
    upool = ctx.enter_context(tc.tile_pool(name="uit", bufs=2))
    spool = ctx.enter_context(tc.tile_pool(name="small", bufs=2))
    pu_pool = ctx.enter_context(tc.tile_pool(name="pu", bufs=2, space="PSUM"))
    pa_pool = ctx.enter_context(tc.tile_pool(name="pa", bufs=2, space="PSUM"))
    py_bufs = 1 if TRANSPOSE == "pe" else 2
    py_pool = ctx.enter_context(tc.tile_pool(name="py", bufs=py_bufs, space="PSUM"))
    ps_pool = ctx.enter_context(tc.tile_pool(name="ps", bufs=1, space="PSUM"))
    if TRANSPOSE == "pe":
        tr_pool = ctx.enter_context(tc.tile_pool(name="tr", bufs=2, space="PSUM"))
    else:
        tr_pool = None

    # Replicated parameters on HWDGE (keeps the gpsimd SWDGE queue free
    # for the x cast-loads). W is consumed as two [128, A] K-chunks.
    W_f = singles.tile([128, 2, A], F32)
    nc.sync.dma_start(W_f[:], W.ap().rearrange("(c k) a -> k c a", c=2))
    W_bf = singles.tile([128, 2, A], BF16)
    nc.vector.tensor_copy(W_bf[:], W_f[:])
    b_sb = singles.tile([A, 1], F32)
    nc.sync.dma_start(b_sb[:], b.ap().rearrange("(a o) -> a o", o=1))
    u_f = singles.tile([A, 1], F32)
    nc.sync.dma_start(u_f[:], u.ap())
    u_bf = singles.tile([A, 1], BF16)
    nc.vector.tensor_copy(u_bf[:], u_f[:])
    ones_f = singles.tile([128, 1], F32)
    nc.vector.memset(ones_f[:], 1.0)
    if TRANSPOSE == "pe":
        from concourse.masks import make_identity

        identity = singles.tile([128, 128], BF16)
        make_identity(nc, identity[:])
    else:
        identity = None

    def one_pass():
        pys = py_pool.tile([BC, 512], F32, tag="py")
        s_pad = spool.tile([128, BC], F32, tag="spad")
        nc.vector.memset(s_pad[:], 0.0)
        b0 = 0
        for nb in LOAD_PLAN:
            xg = xpool.tile([128, nb, I, D], BF16, tag=f"xg{nb}")
            nc.gpsimd.dma_start(
                xg[:],
                x.ap()[b0 : b0 + nb].rearrange("bb (p i) d -> p bb i d", i=I),
            )
            for j in range(nb):
                _emit_batch(tc, pools, params, b0 + j, xg[:, j], pys, s_pad)
            b0 += nb

        # S[b] = sum_p s_pad[p, b] for all batches at once.
        ps8 = ps_pool.tile([BC, 1], F32, tag="ps")
        nc.tensor.matmul(ps8[:], s_pad[:], ones_f[:], start=True, stop=True)

        # y = y' / (S + eps), one DMA out for all batches.
        r_all = spool.tile([BC, 1], F32, tag="rall")
        nc.vector.tensor_scalar_add(r_all[:], ps8[:], EPS)
        nc.vector.reciprocal(r_all[:], r_all[:])
        y_n = spool.tile([BC, D], F32, tag="ynorm")
        nc.vector.tensor_scalar_mul(y_n[:], pys[:, 0:D], r_all[:])
        nc.sync.dma_start(out.ap(), y_n[:])

    pools = (xpool, xtpool, upool, spool, pu_pool, pa_pool, tr_pool)
    params = (W_bf, b_sb, u_bf, ones_f, identity)
    for _ in range(repeat):
        one_pass()


def _emit_batch(tc, pools, params, bi, x_nat, pys, s_pad):
    nc = tc.nc
    (xpool, xtpool, upool, spool, pu_pool, pa_pool, tr_pool) = pools
    (W_bf, b_sb, u_bf, ones_f, identity) = params

    # Transpose: xt[dl, i, ch, cc] = x[t = cc*I + i, d = ch*128 + dl].
    # One tile per chunk so mm1 group g depends only on chunk g.
    n_tr = I // TR_CHUNK
    xts = []
    for tg in range(n_tr):
        xtc = xtpool.tile([128, TR_CHUNK, 2, 128], BF16, tag=f"xt{tg}")
        sl = slice(tg * TR_CHUNK, (tg + 1) * TR_CHUNK)
        if TRANSPOSE == "xbar":
            nc.sync.dma_start(xtc[:], x_nat[:, sl, :], transpose=True)
        else:
            pt = tr_pool.tile([128, TR_CHUNK, 2, 128], BF16, tag="tr")
            for il in range(TR_CHUNK):
                for ch in (0, 1):
                    nc.tensor.transpose(
                        pt[:, il, ch, :],
                        x_nat[:, tg * TR_CHUNK + il, 128 * ch : 128 * (ch + 1)],
                        identity[:],
                    )
            nc.vector.tensor_copy(xtc[:], pt[:])
        xts.append(xtc)

    # mm1 + tanh: uitT[a, i, cc] = tanh(sum_d W[d, a] xT[d, (i cc)] + b[a])
    gi = 4 // TR_CHUNK if TR_CHUNK < 4 else 1  # chunks per 4-i matmul group
    uits = []
    for g in range(I // 4):
        pug = pu_pool.tile([A, 4, 128], F32, tag="pu")
        for ch in (0, 1):
            if TR_CHUNK >= 4:
                xt_g = xts[4 * g // TR_CHUNK]
                off = (4 * g) % TR_CHUNK
                mv = xt_g[:, off : off + 4, ch, :]
                nc.tensor.matmul(
                    pug[:], W_bf[:, ch, :], mv, start=(ch == 0), stop=(ch == 1)
                )
            else:
                for q in range(gi):
                    nc.tensor.matmul(
                        pug[:, q * TR_CHUNK : (q + 1) * TR_CHUNK, :],
                        W_bf[:, ch, :],
                        xts[4 * g // TR_CHUNK + q][:, :, ch, :],
                        start=(ch == 0),
                        stop=(ch == 1),
                    )
        uitg = upool.tile([A, 4, 128], BF16, tag=f"uit{g}")
        nc.scalar.activation(uitg[:], pug[:], TANH, bias=b_sb[:])
        uits.append(uitg)

    # mm2: z[t-part, i] = sum_a uitT[a, i, t-part] * u[a]
    pait = pa_pool.tile([128, I], F32, tag="pa")
    for i in range(I):
        nc.tensor.matmul(
            pait[:, i : i + 1],
            uits[i // 4][:, i % 4, :],
            u_bf[:],
            start=True,
            stop=True,
        )

    # exp -> batch-column bi of a zeroed pad tile (bf16), with fused
    # per-partition row sums into column bi of s_pad (f32).
    e_pad = spool.tile([128, I, BC], BF16, tag="epad")
    nc.vector.memset(e_pad[:], 0.0)
    nc.scalar.activation(
        e_pad[:, :, bi], pait[:], EXP, accum_out=s_pad[:, bi : bi + 1]
    )

    # mm3: row bi of pys accumulates y'_bi = sum_t e[t] x[t, :] (other
    # stationary columns are zero, so other rows get +0).
    for i in range(I):
        nc.tensor.matmul(
            pys[:, 0:D],
            e_pad[:, i, :],
            x_nat[:, i, :],
            start=(bi == 0 and i == 0),
            stop=(bi == BC - 1 and i == I - 1),
        )


_NC_CACHE = {}


def _build_nc(repeat=1, hw_loop=False):
    key = (repeat, hw_loop)
    if key in _NC_CACHE:
        return _NC_CACHE[key]
    nc = bass.Bass()
    x = nc.declare_dram_parameter("x", [BC, T, D], F32, isOutput=False)
    W = nc.declare_dram_parameter("W", [D, A], F32, isOutput=False)
    b = nc.declare_dram_parameter("b", [A], F32, isOutput=False)
    u = nc.declare_dram_parameter("u", [A, 1], F32, isOutput=False)
    out = nc.declare_dram_parameter("out", [BC, D], F32, isOutput=True)
    with tile.TileContext(nc) as tc, ExitStack() as ctx:
        _emit_body(ctx, tc, x, W, b, u, out, repeat=repeat)
    _split_multi_waits(nc)
    _NC_CACHE[key] = nc
    return nc


def make_in_maps(x, W, b, u):
    x = np.ascontiguousarray(x, dtype=np.float32)
    W = np.ascontiguousarray(W, dtype=np.float32)
    b = np.ascontiguousarray(b, dtype=np.float32)
    u = np.ascontiguousarray(u, dtype=np.float32)
    return [
        {"x": x[c * BC : (c + 1) * BC], "W": W, "b": b, "u": u}
        for c in range(N_CORES)
    ]


def kernel(x, W, b, u):
    nc = _build_nc()
    res = run_bass_kernel_spmd(nc, make_in_maps(x, W, b, u), list(range(N_CORES)))
    return np.concatenate([r["out"] for r in res.results], axis=0)
